# revision 6
# baseline (speedup 1.0000x reference)
"""Multi-layer GCN (2x GCNConv + linear head) on 8 Trainium2 NeuronCores.

Strategy (graph/data parallel, node-sharded):
  - Nodes are partitioned contiguously across the 8 cores (6250 each).
  - Each core aggregates messages for its own dst nodes. Edges are bucketed
    by dst tile (128 dsts) on the host, sorted, and padded to 128-edge blocks.
  - Gather of source-node features uses dma_gather (batched indirect DMA)
    from a full node-feature table in HBM. Since dma_gather indices are
    int16, edges are split per tile into src<32768 and src>=32768 groups,
    the latter gathered from an offset view of the table.
  - The scatter-add (segment sum) runs on the TensorEngine: for each
    128-edge block, a one-hot scatter matrix S[e, d] = (dst_e == d) *
    deg_isqrt[dst_e] is built on the VectorEngine with a single
    tensor_scalar(is_equal, mult) against an iota row, and PSUM accumulates
    G_block.T @ S across blocks -> feature-major agg^T tile.
  - deg_isqrt[src] is pre-folded into the gather table rows, so the full
    GCN normalization D^-1/2 (A+I) D^-1/2 comes out of table-scale x S.
  - Layer weights are applied right on the feature-major agg tiles; layer-1
    output is transposed back to node-major (TensorE transpose), scaled by
    deg_isqrt (source-side fold for layer 2) and AllGathered so every core
    has the full h1 table for layer-2 gathers.
  - Layer-2 output stays feature-major and feeds the output projection
    directly (lhsT = h2^T), producing node-major [dst, 64] tiles.
"""

import os
import sys

sys.path.insert(0, "/opt/trn_rl_repo")

import numpy as np

N = int(os.environ.get("GCN_N", 50000))
E = 600000
C_IN = 128
HID = 128
C_OUT = 64
NCORES = 8
NPER = N // NCORES          # 6250
P = 128
NT = (NPER + P - 1) // P    # 49 dst tiles per core
SPLIT = int(os.environ.get("GCN_SPLIT", 32768))  # int16 gather index limit

DTYPE = os.environ.get("GCN_DTYPE", "fp16")  # fp16 | fp32
PHASES = os.environ.get("GCN_PHASES", "full")  # full | A
MAXBLK = int(os.environ.get("GCN_MAXBLK", "1000"))  # max 128-idx blocks per dma_gather call
SHARED = os.environ.get("GCN_SHARED", "1") == "1"

LAST_RESULT = None  # BassKernelResults of the most recent run (for test.py)


def _preprocess(edge_index, x, W1, b1, W2, b2, Wo, bo):
    """Host-side graph preprocessing -> per-core input arrays + schedule."""
    src = np.concatenate([np.asarray(edge_index[0], np.int64), np.arange(N)])
    dst = np.concatenate([np.asarray(edge_index[1], np.int64), np.arange(N)])
    deg = np.bincount(dst, minlength=N).astype(np.float32)
    disqrt = (1.0 / np.sqrt(deg)).astype(np.float32)

    if DTYPE == "fp16":
        tdt, np_tdt = "float16", np.float16
    else:
        tdt, np_tdt = "float32", np.float32

    # gather table: x pre-scaled by src-side normalization
    xs = (np.asarray(x, np.float32) * disqrt[:, None]).astype(np_tdt)

    # per (core, tile) edge buckets
    per_core = []
    nlo = np.zeros((NCORES, NT), np.int64)
    nhi = np.zeros((NCORES, NT), np.int64)
    for c in range(NCORES):
        m = (dst >= c * NPER) & (dst < (c + 1) * NPER)
        s_c = src[m]
        d_c = dst[m] - c * NPER
        order = np.argsort(d_c, kind="stable")
        s_c, d_c = s_c[order], d_c[order]
        bounds = np.searchsorted(d_c, np.arange(0, NT + 1) * P)
        tiles = []
        for t in range(NT):
            ss = s_c[bounds[t]:bounds[t + 1]]
            dd = d_c[bounds[t]:bounds[t + 1]] - t * P
            lo = ss < SPLIT
            tiles.append((ss[lo], dd[lo], ss[~lo], dd[~lo]))
            nlo[c, t] = lo.sum()
            nhi[c, t] = (~lo).sum()
        per_core.append(tiles)

    B_lo = np.maximum(1, -(-nlo.max(axis=0) // P)).astype(np.int64)
    B_hi = np.maximum(1, -(-nhi.max(axis=0) // P)).astype(np.int64)
    NB = int(B_lo.sum() + B_hi.sum())
    tile_ws = [min(P, NPER - t * P) for t in range(NT)]

    in_maps = []
    for c in range(NCORES):
        idx_blk = np.zeros((NB, P), np.int16)
        dstloc = np.zeros((NB, P), np.float32)
        dscale = np.zeros((NB, P), np.float32)
        col = 0
        for t in range(NT):
            ss_lo, dd_lo, ss_hi, dd_hi = per_core[c][t]
            for (ss, dd, nb, base) in (
                (ss_lo, dd_lo, int(B_lo[t]), 0),
                (ss_hi, dd_hi, int(B_hi[t]), SPLIT),
            ):
                n = len(ss)
                flat_i = np.zeros(nb * P, np.int16)
                flat_d = np.zeros(nb * P, np.float32)
                flat_s = np.zeros(nb * P, np.float32)
                flat_i[:n] = (ss - base).astype(np.int16)
                flat_d[:n] = dd.astype(np.float32)
                # dst-side normalization folded into S
                flat_s[:n] = disqrt[dd + t * P + c * NPER]
                idx_blk[col:col + nb] = flat_i.reshape(nb, P)
                dstloc[col:col + nb] = flat_d.reshape(nb, P)
                dscale[col:col + nb] = flat_s.reshape(nb, P)
                col += nb
        assert col == NB
        # wrap indices: per block [128] -> [16, 8] (col s holds idx[16s:16s+16])
        wrapped = idx_blk.reshape(NB, 8, 16).transpose(2, 0, 1).reshape(16, NB * 8)
        idx16 = np.tile(wrapped, (8, 1)).astype(np.int16)

        dsqnm = np.zeros((P, NT), np.float32)
        for t in range(NT):
            tw = tile_ws[t]
            dsqnm[:tw, t] = disqrt[c * NPER + t * P: c * NPER + t * P + tw]

        iota = np.tile(np.arange(P, dtype=np_tdt)[None, :], (P, 1))

        in_maps.append({
            "xs": xs,
            "idx": idx16,
            "dstloc": dstloc.T.copy(),
            "dscale": dscale.T.copy(),
            "dsqnm": dsqnm,
            "iota": iota,
            "w1": np.asarray(W1, np.float32).astype(np_tdt),
            "w2": np.asarray(W2, np.float32).astype(np_tdt),
            "wo": np.asarray(Wo, np.float32).astype(np_tdt),
            "b1": np.asarray(b1, np.float32).reshape(HID, 1).copy(),
            "b2": np.asarray(b2, np.float32).reshape(HID, 1).copy(),
            "bo": np.tile(np.asarray(bo, np.float32)[None, :], (P, 1)),
        })

    sched = dict(B_lo=[int(v) for v in B_lo], B_hi=[int(v) for v in B_hi],
                 NB=NB, tile_ws=tile_ws, tdt=tdt)
    return in_maps, sched


def _build_program(sched):
    import concourse.bass as bass
    import concourse.bacc as bacc
    import concourse.tile as tile
    import concourse.mybir as mybir
    from concourse.masks import make_identity

    f32 = mybir.dt.float32
    i16 = mybir.dt.int16
    tdt = getattr(mybir.dt, sched["tdt"])
    B_lo, B_hi = sched["B_lo"], sched["B_hi"]
    NB, tile_ws = sched["NB"], sched["tile_ws"]

    nc = bacc.Bacc("TRN2", target_bir_lowering=False, debug=False,
                   num_devices=NCORES)

    xs_d = nc.dram_tensor("xs", [N, C_IN], tdt, kind="ExternalInput")
    idx_d = nc.dram_tensor("idx", [P, NB * 8], i16, kind="ExternalInput")
    dstloc_d = nc.dram_tensor("dstloc", [P, NB], f32,
                              kind="ExternalInput")
    dscale_d = nc.dram_tensor("dscale", [P, NB], f32,
                              kind="ExternalInput")
    dsqnm_d = nc.dram_tensor("dsqnm", [P, NT], f32, kind="ExternalInput")
    iota_d = nc.dram_tensor("iota", [P, P], tdt, kind="ExternalInput")
    w1_d = nc.dram_tensor("w1", [C_IN, HID], tdt, kind="ExternalInput")
    w2_d = nc.dram_tensor("w2", [HID, HID], tdt, kind="ExternalInput")
    wo_d = nc.dram_tensor("wo", [HID, C_OUT], tdt, kind="ExternalInput")
    b1_d = nc.dram_tensor("b1", [HID, 1], f32, kind="ExternalInput")
    b2_d = nc.dram_tensor("b2", [HID, 1], f32, kind="ExternalInput")
    bo_d = nc.dram_tensor("bo", [P, C_OUT], f32, kind="ExternalInput")
    out_d = nc.dram_tensor("out", [NPER, C_OUT], f32, kind="ExternalOutput")

    with tile.TileContext(nc) as tc:
        with tc.tile_pool(name="const", bufs=1) as cpool, \
             tc.tile_pool(name="gather", bufs=3) as gpool, \
             tc.tile_pool(name="smat", bufs=8) as spool, \
             tc.tile_pool(name="work", bufs=3) as wpool, \
             tc.tile_pool(name="psA", bufs=2, space="PSUM") as psA, \
             tc.tile_pool(name="psH", bufs=2, space="PSUM") as psH, \
             tc.tile_pool(name="psT", bufs=2, space="PSUM") as psT, \
             tc.tile_pool(name="dram", bufs=1, space="DRAM") as dram:

            def cload(name, dram_t, shape, dt):
                t = cpool.tile(shape, dt, name=name)
                nc.sync.dma_start(t[:], dram_t[tuple(slice(0, s) for s in shape)])
                return t

            idx_sb = cload("idx_sb", idx_d, [P, NB * 8], i16)
            dstloc_sb = cload("dstloc_sb", dstloc_d, [P, NB], f32)
            dscale_sb = cload("dscale_sb", dscale_d, [P, NB], f32)
            dsqnm_sb = cload("dsqnm_sb", dsqnm_d, [P, NT], f32)
            iota_sb = cload("iota_sb", iota_d, [P, P], tdt)
            w1_sb = cload("w1_sb", w1_d, [C_IN, HID], tdt)
            w2_sb = cload("w2_sb", w2_d, [HID, HID], tdt)
            wo_sb = cload("wo_sb", wo_d, [HID, C_OUT], tdt)
            b1_sb = cload("b1_sb", b1_d, [HID, 1], f32)
            b2_sb = cload("b2_sb", b2_d, [HID, 1], f32)
            bo_sb = cload("bo_sb", bo_d, [P, C_OUT], f32)

            ident_sb = cpool.tile([P, P], tdt, name="ident_sb")
            make_identity(nc, ident_sb[:])

            h1s = dram.tile([NPER, HID], tdt, name="h1s")
            h1f = dram.tile([N, HID], tdt, name="h1f",
                            addr_space="Shared" if SHARED else "Local")

            def layer(phase):
                w_sb = w1_sb if phase == 0 else w2_sb
                b_sb = b1_sb if phase == 0 else b2_sb
                col = 0
                for t in range(NT):
                    tw = tile_ws[t]
                    blo, bhi = B_lo[t], B_hi[t]
                    nblk = blo + bhi
                    G = gpool.tile([P, nblk, C_IN], tdt, tag="G", name="G")
                    if phase == 0:
                        tbl_lo, tbl_hi = xs_d[:, :], xs_d[SPLIT:, :]
                    else:
                        tbl_lo, tbl_hi = h1f[:, :], h1f[SPLIT:, :]
                    for (goff, gn, tbl) in ((0, blo, tbl_lo),
                                            (blo, bhi, tbl_hi)):
                        for o in range(0, gn, MAXBLK):
                            nb_call = min(MAXBLK, gn - o)
                            c0 = col + goff + o
                            nc.gpsimd.dma_gather(
                                out_ap=G[:, goff + o:goff + o + nb_call, :],
                                in_ap=tbl,
                                idxs_ap=idx_sb[:, c0 * 8:(c0 + nb_call) * 8],
                                num_idxs=nb_call * P,
                                num_idxs_reg=nb_call * P,
                                elem_size=C_IN)
                    pa = psA.tile([P, tw], f32, tag="pa", name="pa")
                    for j in range(nblk):
                        S = spool.tile([P, tw], tdt, tag="S", name="S")
                        nc.vector.tensor_scalar(
                            out=S[:], in0=iota_sb[:, :tw],
                            scalar1=dstloc_sb[:, col + j:col + j + 1],
                            scalar2=dscale_sb[:, col + j:col + j + 1],
                            op0=mybir.AluOpType.is_equal,
                            op1=mybir.AluOpType.mult)
                        nc.tensor.matmul(pa[:], lhsT=G[:, j, :], rhs=S[:],
                                         start=(j == 0), stop=(j == nblk - 1))
                    col += nblk
                    agg = wpool.tile([P, tw], tdt, tag="agg", name="agg")
                    nc.vector.tensor_copy(agg[:], pa[:])
                    ph = psH.tile([P, tw], f32, tag="ph", name="ph")
                    nc.tensor.matmul(ph[:], lhsT=w_sb[:], rhs=agg[:],
                                     start=True, stop=True)
                    h = wpool.tile([P, tw], tdt, tag="h", name="h")
                    nc.scalar.activation(h[:], ph[:],
                                         mybir.ActivationFunctionType.Relu,
                                         bias=b_sb[:, 0:1])
                    if phase == 0:
                        pt = psT.tile([P, P], tdt, tag="pt", name="pt")
                        nc.tensor.transpose(out=pt[:tw, :], in_=h[:, :tw],
                                            identity=ident_sb[:])
                        hn = wpool.tile([P, P], tdt, tag="hn", name="hn")
                        nc.vector.tensor_scalar(
                            out=hn[:tw, :], in0=pt[:tw, :],
                            scalar1=dsqnm_sb[:tw, t:t + 1], scalar2=None,
                            op0=mybir.AluOpType.mult)
                        nc.sync.dma_start(h1s[t * P:t * P + tw, :], hn[:tw, :])
                    else:
                        po = psT.tile([P, C_OUT], f32, tag="po", name="po")
                        nc.tensor.matmul(po[:tw, :], lhsT=h[:, :tw],
                                         rhs=wo_sb[:], start=True, stop=True)
                        ob = wpool.tile([P, C_OUT], f32, tag="ob", name="ob")
                        nc.vector.tensor_tensor(out=ob[:tw, :], in0=po[:tw, :],
                                                in1=bo_sb[:tw, :],
                                                op=mybir.AluOpType.add)
                        nc.sync.dma_start(out_d[t * P:t * P + tw, :],
                                          ob[:tw, :])

            layer(0)
            if PHASES == "full":
                nc.gpsimd.collective_compute(
                    "AllGather", mybir.AluOpType.bypass,
                    replica_groups=[list(range(NCORES))],
                    ins=[h1s[:].opt()], outs=[h1f[:].opt()])
                layer(1)
            else:
                ob = wpool.tile([P, C_OUT], f32, tag="ob", name="ob")
                nc.gpsimd.memset(ob[:], 0.0)
                for t in range(NT):
                    tw = tile_ws[t]
                    nc.sync.dma_start(out_d[t * P:t * P + tw, :], ob[:tw, :])

    nc.compile()
    return nc


def kernel(x, edge_index, W1, b1, W2, b2, Wo, bo):
    global LAST_RESULT
    from concourse import bass_utils

    in_maps, sched = _preprocess(edge_index, x, W1, b1, W2, b2, Wo, bo)
    nc = _build_program(sched)
    res = bass_utils.run_bass_kernel_spmd(nc, in_maps,
                                          core_ids=list(range(NCORES)))
    LAST_RESULT = res
    out = np.concatenate([res.results[c]["out"] for c in range(NCORES)], axis=0)
    return out.astype(np.float32)
